# revision 2
# baseline (speedup 1.0000x reference)
"""Trainium2 Bass kernel for nn_EmbeddingLayer (embedding lookup + dense + positional encoding).

Computation (per reference):
    x = emb_table[tokens] * sqrt(512)          [B, F, E]
    x = x.reshape(B, F*E) @ W + b              [B, M]
    out = x[:, None, :] + pe[:128, :]          [B, S, M]   (1 GiB f32 output)

Strategy: data-parallel over batch across 8 cores (512 rows each); the
74 MB table and the 4 MB dense weight are replicated.  Per core:
  - indirect-DMA gather of 512*64 embedding rows into [128b, 2048] tiles
  - PE transpose -> xT, f32 matmul against pre-scaled W (scale and bias
    folded on host into W / pe)
  - pe rows broadcast across partitions with a ones(1x128) matmul on PE
  - DVE broadcast-add y[b,m] + pe[s,m] -> [128, 8*512] tiles
  - 2 MiB HWDGE stores of the 128 MiB per-core output (the roofline)
"""

import sys

import numpy as np

if "/opt/trn_rl_repo" not in sys.path:
    sys.path.insert(0, "/opt/trn_rl_repo")

BATCH = 4096
FEATURES = 64
VOCAB = 580000
EMB = 32
MODELS = 512
SEQ = 128
N_CORES = 8
BS = BATCH // N_CORES  # 512 rows per core

P = 128                 # partitions
NB = BS // P            # 4 batch chunks per core
KC = (FEATURES * EMB) // P  # 16 k-chunks of 128
G = 8                   # seq positions per output tile
NG = SEQ // G           # 16 seq groups

_MODULE_CACHE = {}


def _positional_encoding(position, d_model):
    # mirror of reference._positional_encoding, in numpy f32
    pos = np.arange(position, dtype=np.float32)[:, None]
    i = np.arange(d_model, dtype=np.float32)[None, :]
    angle_rates = 1.0 / np.power(
        10000.0, (2.0 * np.floor(i / 2.0)) / np.float32(d_model)
    )
    angles = (pos * angle_rates).astype(np.float32)
    even = (np.arange(d_model) % 2 == 0)[None, :]
    pe = np.where(even, np.sin(angles), np.cos(angles)).astype(np.float32)
    return pe  # [S, M]


def build_module():
    """Build + compile the per-core Bass module (identical program on all cores)."""
    if "nc" in _MODULE_CACHE:
        return _MODULE_CACHE["nc"]

    from contextlib import ExitStack

    import concourse.bass as bass
    import concourse.tile as tile
    from concourse import bacc, mybir

    f32 = mybir.dt.float32
    i32 = mybir.dt.int32

    nc = bacc.Bacc("TRN2", target_bir_lowering=False, debug=False,
                   num_devices=N_CORES)

    tok = nc.dram_tensor("tok", [BS, FEATURES], i32, kind="ExternalInput").ap()
    emb = nc.dram_tensor("emb", [VOCAB, EMB], f32, kind="ExternalInput").ap()
    w = nc.dram_tensor("w", [FEATURES * EMB, MODELS], f32, kind="ExternalInput").ap()
    pe = nc.dram_tensor("pe", [SEQ, MODELS], f32, kind="ExternalInput").ap()
    ident = nc.dram_tensor("ident", [P, P], f32, kind="ExternalInput").ap()
    ones = nc.dram_tensor("ones", [1, P], f32, kind="ExternalInput").ap()
    out = nc.dram_tensor("out", [BS, SEQ, MODELS], f32, kind="ExternalOutput").ap()

    with tile.TileContext(nc) as tc, ExitStack() as ctx:
        const = ctx.enter_context(tc.tile_pool(name="const", bufs=1))
        tok_pool = ctx.enter_context(tc.tile_pool(name="tok", bufs=2))
        x_pool = ctx.enter_context(tc.tile_pool(name="x", bufs=2))
        xT_pool = ctx.enter_context(tc.tile_pool(name="xT", bufs=2))
        y_pool = ctx.enter_context(tc.tile_pool(name="y", bufs=NB))
        perow_pool = ctx.enter_context(tc.tile_pool(name="perow", bufs=2))
        pegrp_pool = ctx.enter_context(tc.tile_pool(name="pegrp", bufs=2))
        out_pool = ctx.enter_context(tc.tile_pool(name="outp", bufs=3))
        psum_t = ctx.enter_context(tc.tile_pool(name="pst", bufs=2, space="PSUM"))
        psum_y = ctx.enter_context(tc.tile_pool(name="psy", bufs=2, space="PSUM"))
        psum_p = ctx.enter_context(tc.tile_pool(name="psp", bufs=2, space="PSUM"))

        # constants
        w_sb = const.tile([P, KC * MODELS], f32)
        nc.sync.dma_start(
            w_sb[:].rearrange("p (kc m) -> p kc m", kc=KC),
            w.rearrange("(kc p) m -> p kc m", p=P),
        )
        id_sb = const.tile([P, P], f32)
        nc.sync.dma_start(id_sb[:], ident[:])
        ones_sb = const.tile([1, P], f32)
        nc.sync.dma_start(ones_sb[:], ones[:])

        # per-chunk: gather -> transpose -> matmul -> y
        ys = []
        for c in range(NB):
            tok_sb = tok_pool.tile([P, FEATURES], i32)
            nc.sync.dma_start(tok_sb[:], tok[c * P:(c + 1) * P, :])

            # HW indirect DMA honors one index per partition: issue one gather
            # per token column (128 rows x 128B each).
            x_sb = x_pool.tile([P, FEATURES * EMB], f32)
            for f in range(FEATURES):
                nc.gpsimd.indirect_dma_start(
                    out=x_sb[:, f * EMB:(f + 1) * EMB],
                    out_offset=None,
                    in_=emb[:],
                    in_offset=bass.IndirectOffsetOnAxis(
                        ap=tok_sb[:, f:f + 1], axis=0
                    ),
                )

            xT_sb = xT_pool.tile([P, KC * P], f32)
            for kc in range(KC):
                pt = psum_t.tile([P, P], f32)
                nc.tensor.transpose(
                    out=pt[:], in_=x_sb[:, kc * P:(kc + 1) * P], identity=id_sb[:]
                )
                nc.vector.tensor_copy(xT_sb[:, kc * P:(kc + 1) * P], pt[:])

            py = psum_y.tile([P, MODELS], f32)
            for kc in range(KC):
                nc.tensor.matmul(
                    py[:],
                    lhsT=xT_sb[:, kc * P:(kc + 1) * P],
                    rhs=w_sb[:, kc * MODELS:(kc + 1) * MODELS],
                    start=(kc == 0),
                    stop=(kc == KC - 1),
                )
            y_sb = y_pool.tile([P, MODELS], f32)
            nc.scalar.copy(y_sb[:], py[:])
            ys.append(y_sb)

        # per seq-group: broadcast pe rows across partitions, add, store
        for g in range(NG):
            perow = perow_pool.tile([1, G * MODELS], f32)
            nc.sync.dma_start(
                perow[:].rearrange("p (g m) -> p g m", g=G),
                pe[g * G:(g + 1) * G, :].unsqueeze(0),
            )
            peg = pegrp_pool.tile([P, G * MODELS], f32)
            for sl in range(G):
                pp = psum_p.tile([P, MODELS], f32)
                nc.tensor.matmul(
                    pp[:],
                    lhsT=ones_sb[:],
                    rhs=perow[:, sl * MODELS:(sl + 1) * MODELS],
                    start=True,
                    stop=True,
                )
                nc.scalar.copy(peg[:, sl * MODELS:(sl + 1) * MODELS], pp[:])

            for c in range(NB):
                ot = out_pool.tile([P, G * MODELS], f32)
                nc.vector.tensor_tensor(
                    out=ot[:].rearrange("p (g m) -> p g m", g=G),
                    in0=ys[c][:].unsqueeze(1).to_broadcast([P, G, MODELS]),
                    in1=peg[:].rearrange("p (g m) -> p g m", g=G),
                    op=mybir.AluOpType.add,
                )
                nc.sync.dma_start(
                    out[c * P:(c + 1) * P, g * G:(g + 1) * G, :],
                    ot[:].rearrange("p (g m) -> p g m", g=G),
                )

    nc.compile()
    _MODULE_CACHE["nc"] = nc
    return nc


def make_in_maps(tokens, emb_table, W, b):
    tokens = np.ascontiguousarray(np.asarray(tokens, dtype=np.int32))
    emb_table = np.ascontiguousarray(np.asarray(emb_table, dtype=np.float32))
    W = np.asarray(W, dtype=np.float32)
    b = np.asarray(b, dtype=np.float32)

    wp = np.ascontiguousarray(W * np.float32(np.sqrt(np.float32(MODELS))))
    peb = np.ascontiguousarray(
        _positional_encoding(SEQ, MODELS) + b[None, :].astype(np.float32)
    )
    ident = np.eye(P, dtype=np.float32)
    ones = np.ones((1, P), dtype=np.float32)

    in_maps = []
    for c in range(N_CORES):
        in_maps.append({
            "tok": tokens[c * BS:(c + 1) * BS],
            "emb": emb_table,
            "w": wp,
            "pe": peb,
            "ident": ident,
            "ones": ones,
        })
    return in_maps


def run(tokens, emb_table, W, b, trace=False):
    """Run on 8 NeuronCores; returns (full_output, BassKernelResults)."""
    from concourse import bass_utils

    nc = build_module()
    in_maps = make_in_maps(tokens, emb_table, W, b)
    res = bass_utils.run_bass_kernel_spmd(
        nc, in_maps, core_ids=list(range(N_CORES)), trace=trace
    )
    outs = [r["out"] for r in res.results]
    full = np.concatenate(outs, axis=0)
    return full, res


def kernel(tokens, emb_table, W, b):
    full, _ = run(tokens, emb_table, W, b, trace=False)
    return full
